# revision 23
# baseline (speedup 1.0000x reference)
"""Conv2d 3x3 (stride 1, pad 1) forward on 8 Trainium2 NeuronCores.

v8: fold the 3 kw-taps into the matmul contraction dim (K=96) so each
round needs 3 passes instead of 9; run the 2 images of a pair
concurrently on the two column halves of the PE array.

The 129-pitch trick: each image row is stored as [z | x row] with
pitch 129 (z = zero).  Position (r, 129) aliases (r+1, 0), so the one
zero column is BOTH the right halo of row r and the left halo of row
r+1.  Replica group g for tap kw=g satisfies G_g[f] = L[f+g] -- a
flat element shift.

The HOST pre-builds BOTH L and G1 (input DMA 2x but fully contiguous
and port-balanced: L lands on odd-port partitions 64-95, G1 on
even-port 0-31).  Only G2 is built on-device: VectorE flat shifted
copy G2 = G1 << 1 (parts 32-63), chunk-pipelined one row behind the
input stream.  Every AP slices a single image -- an AP spanning both
images bounds nearly the whole tile and poisons the dependency
tracker with false WAR/RAW edges (cost: 30 us of serialization).

Both pairs share one partition layout (G1@0-31, G2@32-63, L@64-95,
weight rows permuted (kw1,kw2,kw0)), so one stationary weight tile
serves all matmuls; rounds issue kh-major so consecutive matmuls on
a PE column tile reuse the loaded stationary.

Per round (4 output rows) and pair: 3 matmuls (kh=0..2) of K=96 x
N=512 accumulate into PSUM [128, 4, 128] (two images = two column
tiles); ScalarE drains with fused bias (VectorE takes late-round
drains once its copies finish); bf16 staging, output DMA batched 4
rounds deep (2 MiB per store).
"""
import sys
sys.path.insert(0, '/opt/trn_rl_repo')
import numpy as np
import ml_dtypes

BF16 = ml_dtypes.bfloat16
B, Cin, H, W = 32, 32, 128, 128
Cout, KH, KW = 64, 3, 3
NCORES = 8
BPC = B // NCORES          # images per core
NPAIR = 2
Hp = H + 2
PITCH = W + 1              # 129: [zero col | x row]
SZ = Hp * PITCH            # flat elems per image
FLAT = 2 * SZ + 4          # + tail pad (shifted copies read past the end)
R = 4                      # output rows per round
NROUND = H // R
RB = 4                     # rounds per output DMA batch
NCH = 8                    # input chunks
ROWS_CH = H // NCH
PERM = (1, 2, 0)           # partition-group -> kw (both pairs)

_cache = {}


def _build_program():
    from concourse import bacc
    import concourse.mybir as mybir
    from concourse.tile import TileContext

    f32 = mybir.dt.float32
    bf16 = mybir.dt.bfloat16
    Act = mybir.ActivationFunctionType

    nc = bacc.Bacc("TRN2", target_bir_lowering=False, debug=False,
                   num_devices=NCORES)
    x_ext = nc.declare_dram_parameter("x", [NPAIR, 2, Cin, 2, Hp, PITCH],
                                      bf16, isOutput=False)
    w_ext = nc.declare_dram_parameter("w", [128, KH, Cout], bf16,
                                      isOutput=False)
    b_ext = nc.declare_dram_parameter("b", [128, 1], f32, isOutput=False)
    out_ext = nc.declare_dram_parameter("out", [BPC * Cout, H, W], bf16,
                                        isOutput=True)

    with TileContext(nc) as tc:
        with tc.tile_pool(name="xq", bufs=1) as xpool, \
             tc.tile_pool(name="const", bufs=1) as cpool, \
             tc.tile_pool(name="stage", bufs=3) as opool, \
             tc.tile_pool(name="psum", bufs=8, space="PSUM") as ppool:

            xq = [xpool.tile([128, FLAT], bf16, name=f"xq{p}")
                  for p in range(NPAIR)]
            v = [t[:, 0:2 * SZ].rearrange("p (im r c) -> p im r c",
                                          im=2, r=Hp, c=PITCH)
                 for t in xq]
            # shift-1 alias: vs1[p] reads element f+1 where v reads f
            vs1 = [t[:, 1:1 + 2 * SZ].rearrange("p (im r c) -> p im r c",
                                                im=2, r=Hp, c=PITCH)
                   for t in xq]
            wt = cpool.tile([128, KH, Cout], bf16)
            bt = cpool.tile([128, 1], f32)

            nc.sync.dma_start(out=wt[:], in_=w_ext[:])
            nc.sync.dma_start(out=bt[:], in_=b_ext[:])

            for p in range(NPAIR):
                # tail pad after G1 (the shifted copy reads past its end)
                nc.vector.memset(xq[p][0:32, 2 * SZ:FLAT], 0.0)

            # dest partition base: rep 0 (L) -> 64 (odd DMA ports),
            # rep 1 (G1) -> 0 (even ports)
            rbase = (64, 0)
            for g in range(NCH):
                rs = 0 if g == 0 else 1 + g * ROWS_CH
                re = Hp if g == NCH - 1 else 1 + (g + 1) * ROWS_CH
                for p in range(NPAIR):
                    for rep in range(2):
                        b0 = rbase[rep]
                        for im in range(2):
                            nc.sync.dma_start(
                                out=v[p][b0:b0 + 32, im, rs:re, :],
                                in_=x_ext[p, rep, :, im, rs:re, :])
                # G2 = G1 shifted by one element, lagged one row behind
                # the input chunk so the 2-element source overhang is
                # already written
                crs = max(0, rs - 1)
                cre = Hp if g == NCH - 1 else re - 1
                for p in range(NPAIR):
                    for im in range(2):
                        nc.vector.tensor_copy(
                            v[p][32:64, im, crs:cre, :],
                            vs1[p][0:32, im, crs:cre, :])

            out_v = out_ext.rearrange(
                "(ip half co) h w -> (half co) ip (h w)",
                ip=2, half=2, co=Cout)

            for k in range(NROUND):
                h0 = k * R
                if k % RB == 0:
                    ost = opool.tile([128, 2, RB * R, W], bf16, tag="ost")
                roff = (k % RB) * R
                ps = [ppool.tile([128, R, W], f32, tag="ps",
                                 name=f"ps{k}_{p}") for p in range(NPAIR)]
                # kh-major so the stationary weight tile is reused by all
                # four matmuls (2 pairs x 2 images) of the same kh
                for kh in range(KH):
                    for p in range(NPAIR):
                        for im in range(2):
                            nc.tensor.matmul(
                                ps[p][64 * im:64 * im + 64, :, :],
                                wt[0:96, kh, :],
                                v[p][0:96, im, h0 + kh:h0 + kh + R, 0:W],
                                start=(kh == 0), stop=(kh == KH - 1),
                                tile_position=(0, 64 * im))
                for p in range(NPAIR):
                    if p == 1 and k >= 20:
                        nc.vector.tensor_scalar_add(
                            ost[:, p, roff:roff + R, :], ps[p][:, :, :],
                            bt[:, :])
                    else:
                        nc.scalar.activation(ost[:, p, roff:roff + R, :],
                                             ps[p][:, :, :], Act.Identity,
                                             bias=bt[:, :])
                if k % RB == RB - 1:
                    hb = (k - (RB - 1)) * R
                    nc.sync.dma_start(
                        out=out_v[:, :, hb * W:(hb + RB * R) * W],
                        in_=ost[:, :, :, :])

    nc.compile()
    return nc


def _get_program():
    if "nc" not in _cache:
        _cache["nc"] = _build_program()
    return _cache["nc"]


def _prep_inputs(x, kernel, bias):
    # weights -> [32*g + ci, kh, co] with group g holding kw = PERM[g]
    kr = kernel.reshape(Cout, Cin, KH, KW).astype(np.float32)
    w = np.zeros((128, KH, Cout), dtype=np.float32)
    for g in range(3):
        w[32 * g:32 * g + 32] = np.transpose(kr[:, :, :, PERM[g]], (1, 2, 0))
    w = np.ascontiguousarray(w).astype(BF16)
    b = np.ascontiguousarray(np.tile(bias.astype(np.float32), 2)[:, None])
    # host pre-builds the 129-pitch L layout AND its flat-shift G1
    xb = x.astype(BF16)
    in_maps = []
    for c in range(NCORES):
        xs = xb[c * BPC:(c + 1) * BPC]          # [4, Cin, H, W]
        L = np.zeros((NPAIR, Cin, 2, Hp, PITCH), dtype=BF16)
        for p in range(NPAIR):
            for im in range(2):
                L[p, :, im, 1:1 + H, 1:PITCH] = xs[2 * p + im]
        Lf = L.reshape(NPAIR, Cin, 2, SZ)
        G1f = np.zeros_like(Lf)
        G1f[..., :-1] = Lf[..., 1:]
        X = np.stack([Lf, G1f], axis=1).reshape(
            NPAIR, 2, Cin, 2, Hp, PITCH)
        in_maps.append({"x": np.ascontiguousarray(X), "w": w, "b": b})
    return in_maps


def _run(inputs, trace=False):
    from concourse.bass_utils import run_bass_kernel_spmd
    nc = _get_program()
    in_maps = _prep_inputs(inputs["x"], inputs["kernel"], inputs["bias"])
    res = run_bass_kernel_spmd(nc, in_maps, list(range(NCORES)), trace=trace)
    out = np.concatenate(
        [res.results[c]["out"].reshape(BPC, Cout, H, W)
         for c in range(NCORES)], axis=0)
    return out.astype(np.float32), res


def kernel(**inputs):
    out, _ = _run(inputs, trace=False)
    return out
